# revision 12
# baseline (speedup 1.0000x reference)
"""Trainium2 Bass kernel for nn_Conv2d_24833500905755 (3x3 conv, B=32,
C_in=64, C_out=128, 56x56, pad 1, with the reference's mismatched
weight-flatten order).

Math: out[b,co,h,w] = sum_{c,di,dj} xpad[b,c,h+di,w+dj] * Wt[c,di*3+dj,co]
with Wt = K.reshape(576, C_OUT).reshape(C_IN, 9, C_OUT).

Data-parallel: 4 images per NeuronCore, 2 images packed on the
128-partition dim (fp16 matmuls, K=64 contraction per half, concurrent
PE row-group tiles). Raw-bass hand-scheduled engine programs.

v4 scheduling model (calibrated from perfetto traces):
  - DMA chain: issue (DIRECT2D ~0.62us on HWDGE engines, ~25ns on Pool)
    + hwdge/dge ~1.3us + transfer (360 GB/s aggregate, fair-shared)
    + 0.9us semaphore propagation.  Critical first pieces are issued
    PRE-BLOCK from the four earliest engine slots so the real matmul
    stream can start at ~9.8us instead of ~11.5.
  - PE p-state: MID (2x slow) until ~6us of execution; resets on idle.
    Junk matmuls bridge from sequencer start (~7.6us) to first-data
    (~9.8us) so the stream never gaps.
  - Tail: last-chunk copy + DMA chain is exposed (~4.5us); pair 1 ends
    with two 4-row minichunks and per-chunk output DMAs to shrink the
    exposed copy+transfer and decongest the final queues.

Engine programs:
  pre-block: Vector: w taps 0-3; Scalar: w taps 4-8; GpSimd: xA rows
             [0,10); Sync: xB rows [10,34); Tensor: 1 probe junk
  Sync:   xC rows [34,58); pair-0/1 half-0 output DMAs; final wait
  Scalar: half-1 PSUM->SBUF copies (fp32->fp16), pair-0/1 half-1 DMAs
  GpSimd: pair-1 input DMAs at s_mm>=2
  Tensor: junk ramp bridge, then 252 fp16 matmuls + 36 minichunk mms
  Vector: half-0 PSUM->SBUF copies (fp32 -> fp16)

Output is fp16 on-chip and in HBM; host upcasts to fp32.
"""

from contextlib import ExitStack

import numpy as np

import concourse.bass as bass
import concourse.mybir as mybir
from concourse.bass_utils import run_bass_kernel_spmd

B, C_IN, C_OUT, H = 32, 64, 128, 56
KS = 3
N_CORES = 8
BPC = B // N_CORES
HP = H + 2
RCHUNK = 8
NCHUNK = H // RCHUNK          # 7 chunks/image, 14 global chunks (2 pairs)
# pair-0 output blocks (8-row chunks grouped); pair-1 gets finer blocks
OBLOCKS0 = [(0, 24), (24, 40), (40, 48), (48, 56)]
OBLOCKS1 = [(0, 24), (24, 32), (32, 40), (40, 48)]  # + minichunks (48,52),(52,56)
MM_DT = mybir.dt.float16
OUT_DT = mybir.dt.float16
N_JUNK_PRE = 1                # pre-block probe junk (also earliest ramp start)
N_JUNK = 10                   # in-block 448-col junks (5 pair-slots)
N_JUNK_SMALL = 6              # in-block 112-col tail junks (3 pair-slots)


def _block_of(blocks, h0):
    for bi, (blo, bhi) in enumerate(blocks):
        if blo <= h0 < bhi:
            return bi, blo, bhi
    raise AssertionError(h0)


def build_nc(mm_dt=MM_DT):
    f32 = mybir.dt.float32
    nc = bass.Bass()
    x_ext = nc.declare_dram_parameter("x", [BPC, C_IN, HP, HP], mm_dt, isOutput=False)
    w_ext = nc.declare_dram_parameter("w", [2 * C_IN, KS * KS, C_OUT], mm_dt, isOutput=False)
    out_ext = nc.declare_dram_parameter("out", [BPC, C_OUT, H, H], OUT_DT, isOutput=True)

    # out DMAs: pair0 4 blocks x2 halves + pair1 (4 blocks + 2 minis) x2
    n_out_dmas = len(OBLOCKS0) * 2 + (len(OBLOCKS1) + 2) * 2

    with ExitStack() as ctx:
        wt = ctx.enter_context(nc.sbuf_tensor("wt", [2 * C_IN, KS * KS, C_OUT], mm_dt))
        xps = [
            ctx.enter_context(nc.sbuf_tensor(f"xp{p}", [2 * C_IN, HP, HP], mm_dt))
            for p in range(2)
        ]
        # obs[p][half][block]; pair-1 block 'last' covers rows 48-56 split
        # into two 4-row DMA pieces
        obs = [
            [
                [
                    ctx.enter_context(
                        nc.sbuf_tensor(f"ob_{p}_{h}_{bi}", [C_OUT, bhi - blo, H], OUT_DT)
                    )
                    for bi, (blo, bhi) in enumerate(
                        OBLOCKS0 if p == 0 else OBLOCKS1 + [(48, 52), (52, 56)]
                    )
                ]
                for h in range(2)
            ]
            for p in range(2)
        ]
        # banks[slot][half] - 8 full PSUM banks + 2 minibanks for the tail
        banks = [
            [
                ctx.enter_context(
                    nc.psum_tensor(f"ps_{s}_{h}", [C_OUT, RCHUNK, H], f32)
                )
                for h in range(2)
            ]
            for s in range(4)
        ]
        # minichunk banks: subranges of retired full banks (PSUM is fully
        # allocated). mini0 -> banks[3] (last used by chunk 11), mini1 ->
        # banks[1] (chunk 9); explicit WAR waits before reuse.
        minib = [banks[3][h][:, 0:4, :] for h in range(2)]
        s_w = ctx.enter_context(nc.semaphore("s_w"))
        s_x = [ctx.enter_context(nc.semaphore(f"s_x{p}")) for p in range(2)]
        s_xa = ctx.enter_context(nc.semaphore("s_xa"))
        s_mm = ctx.enter_context(nc.semaphore("s_mm"))
        s_cp = ctx.enter_context(nc.semaphore("s_cp"))
        s_cp2 = ctx.enter_context(nc.semaphore("s_cp2"))
        s_out = ctx.enter_context(nc.semaphore("s_out"))

        src0 = x_ext[0:2].rearrange("b c h w -> (b c) h w")
        src1 = x_ext[2:4].rearrange("b c h w -> (b c) h w")

        # ---- pre-block critical DMA issues (earliest engine slots) ----
        # (DMA-capable engines: sync/SP, scalar/Activation, gpsimd/Pool)
        nc.sync.dma_start(out=wt[:, 0:4, :], in_=w_ext[:, 0:4, :]).then_inc(s_w, 16)
        nc.scalar.dma_start(out=wt[:, 4:, :], in_=w_ext[:, 4:, :]).then_inc(s_w, 16)
        nc.gpsimd.dma_start(out=xps[0][:, 0:10, :], in_=src0[:, 0:10, :]).then_inc(s_xa, 16)
        nc.scalar.dma_start(out=xps[0][:, 10:34, :], in_=src0[:, 10:34, :]).then_inc(s_x[0], 16)
        # pre-block probe junk: measures earliest PE dispatch + starts ramp
        for wi in range(N_JUNK_PRE):
            nc.tensor.matmul(
                out=banks[3][0][:],
                lhsT=wt[0:C_IN, 0, :],
                rhs=xps[0][0:C_IN, 0:RCHUNK, 0:H],
                start=True,
                stop=True,
            )

        with nc.Block() as block:

            @block.sync
            def _(sync: bass.BassEngine):
                sync.dma_start(out=xps[0][:, 34:HP, :], in_=src0[:, 34:HP, :]).then_inc(s_x[0], 16)
                for p in range(2):
                    blocks = OBLOCKS0 if p == 0 else OBLOCKS1 + [(48, 52), (52, 56)]
                    for bi, (blo, bhi) in enumerate(blocks):
                        # copies tick s_cp once per 8-row chunk and once per
                        # 4-row minichunk (pair-1 rows 48-56 = 2 ticks)
                        if p == 0:
                            ticks = bhi // RCHUNK
                        elif bhi <= 48:
                            ticks = NCHUNK + bhi // RCHUNK
                        else:
                            ticks = 14 if bhi == 52 else 15
                        sync.wait_ge(s_cp, ticks)
                        dst = out_ext[2 * p : 2 * p + 1].rearrange("b c h w -> (b c) h w")
                        sync.dma_start(
                            out=dst[:, blo:bhi, :], in_=obs[p][0][bi][:]
                        ).then_inc(s_out, 16)
                sync.wait_ge(s_out, 16 * n_out_dmas)

            @block.scalar
            def _(scalar: bass.BassEngine):
                cp2 = 0
                for p in range(2):
                    blocks = OBLOCKS0 if p == 0 else OBLOCKS1
                    for ci in range(NCHUNK):
                        c = p * NCHUNK + ci
                        h0 = ci * RCHUNK
                        if p == 1 and ci == NCHUNK - 1:
                            break  # handled as minichunks below
                        bi, blo, bhi = _block_of(blocks, h0)
                        scalar.wait_ge(s_mm, 2 * (c + 1))
                        scalar.copy(
                            out=obs[p][1][bi][:, h0 - blo : h0 - blo + RCHUNK, :],
                            in_=banks[c % 4][1][:],
                        ).then_inc(s_cp2, 1)
                        cp2 += 1
                        if h0 + RCHUNK == bhi:
                            scalar.wait_ge(s_cp2, cp2)
                            dst = out_ext[2 * p + 1 : 2 * p + 2].rearrange(
                                "b c h w -> (b c) h w"
                            )
                            scalar.dma_start(
                                out=dst[:, blo:bhi, :], in_=obs[p][1][bi][:]
                            ).then_inc(s_out, 16)
                # pair-1 minichunks: rows 48-52 (minib), 52-56 (banks[1][:, 0:4])
                dst = out_ext[3:4].rearrange("b c h w -> (b c) h w")
                for mc, bank in enumerate([minib[1], banks[1][1][:, 0:4, :]]):
                    bi = len(OBLOCKS1) + mc
                    blo = 48 + 4 * mc
                    scalar.wait_ge(s_mm, 26 + 2 * (mc + 1))
                    scalar.copy(out=obs[1][1][bi][:], in_=bank).then_inc(s_cp2, 1)
                    cp2 += 1
                    scalar.wait_ge(s_cp2, cp2)
                    scalar.dma_start(
                        out=dst[:, blo : blo + 4, :], in_=obs[1][1][bi][:]
                    ).then_inc(s_out, 16)

            @block.gpsimd
            def _(gpsimd: bass.BassEngine):
                gpsimd.wait_ge(s_mm, 2)
                gpsimd.dma_start(out=xps[1][:, 0:12, :], in_=src1[:, 0:12, :]).then_inc(s_x[1], 16)
                gpsimd.dma_start(out=xps[1][:, 12:34, :], in_=src1[:, 12:34, :]).then_inc(s_x[1], 16)
                gpsimd.dma_start(out=xps[1][:, 34:HP, :], in_=src1[:, 34:HP, :]).then_inc(s_x[1], 16)

            @block.tensor
            def _(tensor: bass.BassEngine):
                # Junk ramp bridge: sequencer start (~7.6us) to first data
                # (~9.8us); banks[3] first reused by chunk 3 (start=True).
                for wi in range(N_JUNK):
                    h = wi % 2
                    c0 = h * C_IN
                    tensor.matmul(
                        out=banks[3][h][:],
                        lhsT=wt[c0 : c0 + C_IN, 0, :],
                        rhs=xps[0][c0 : c0 + C_IN, 0:RCHUNK, 0:H],
                        start=True,
                        stop=True,
                    )
                for wi in range(N_JUNK_SMALL):
                    h = wi % 2
                    c0 = h * C_IN
                    tensor.matmul(
                        out=banks[3][h][:, 0:2, :],
                        lhsT=wt[c0 : c0 + C_IN, 0, :],
                        rhs=xps[0][c0 : c0 + C_IN, 0:2, 0:H],
                        start=True,
                        stop=True,
                    )
                for p in range(2):
                    for ci in range(NCHUNK):
                        c = p * NCHUNK + ci
                        h0 = ci * RCHUNK
                        if p == 1 and ci == NCHUNK - 1:
                            break  # minichunks below
                        if p == 0:
                            if ci == 0:
                                tensor.wait_ge(s_w, 16)   # taps 0-3
                                tensor.wait_ge(s_xa, 16)  # rows [0,10)
                            elif ci == 1:
                                tensor.wait_ge(s_x[0], 16)  # rows [10,34)
                            elif ci == 4:
                                tensor.wait_ge(s_x[0], 32)  # rows [34,58)
                        else:
                            if ci == 0:
                                tensor.wait_ge(s_x[1], 16)
                            elif ci == 1:
                                tensor.wait_ge(s_x[1], 32)
                            elif ci == 4:
                                tensor.wait_ge(s_x[1], 48)
                        if c >= 4:
                            # WAR: bank slot c%4 last used by chunk c-4
                            tensor.wait_ge(s_cp, c - 3)
                            tensor.wait_ge(s_cp2, c - 3)
                        for k in range(KS * KS):
                            di, dj = divmod(k, KS)
                            last = k == KS * KS - 1
                            if p == 0 and ci == 0 and k == 4:
                                tensor.wait_ge(s_w, 32)  # taps 4-8
                            for half in range(2):
                                c0 = half * C_IN
                                mm = tensor.matmul(
                                    out=banks[c % 4][half][:],
                                    lhsT=wt[c0 : c0 + C_IN, k, :],
                                    rhs=xps[p][
                                        c0 : c0 + C_IN,
                                        h0 + di : h0 + di + RCHUNK,
                                        dj : dj + H,
                                    ],
                                    start=(k == 0),
                                    stop=last,
                                )
                                if last and half == 1:
                                    mm.then_inc(s_mm, 2)
                # pair-1 rows 48-56 as two 4-row minichunks (shrinks the
                # exposed tail copy+DMA). minib fresh; banks[1] last used by
                # chunk 9 (copied long before ~34us).
                for mc in range(2):
                    h0 = 48 + 4 * mc
                    if mc == 0:
                        # WAR: banks[3] last written by chunk 11 (tick 12),
                        # banks[1] by chunk 9 (tick 10) — both long done
                        tensor.wait_ge(s_cp, 12)
                        tensor.wait_ge(s_cp2, 12)
                    for k in range(KS * KS):
                        di, dj = divmod(k, KS)
                        last = k == KS * KS - 1
                        for half in range(2):
                            c0 = half * C_IN
                            out_bank = (
                                minib[half] if mc == 0 else banks[1][half][:, 0:4, :]
                            )  # minib aliases banks[3][:, 0:4]
                            mm = tensor.matmul(
                                out=out_bank,
                                lhsT=wt[c0 : c0 + C_IN, k, :],
                                rhs=xps[1][
                                    c0 : c0 + C_IN,
                                    h0 + di : h0 + di + 4,
                                    dj : dj + H,
                                ],
                                start=(k == 0),
                                stop=last,
                            )
                            if last and half == 1:
                                mm.then_inc(s_mm, 2)

            @block.vector
            def _(vector: bass.BassEngine):
                cp = 0
                for p in range(2):
                    blocks = OBLOCKS0 if p == 0 else OBLOCKS1
                    for ci in range(NCHUNK):
                        c = p * NCHUNK + ci
                        h0 = ci * RCHUNK
                        if p == 1 and ci == NCHUNK - 1:
                            break
                        bi, blo, bhi = _block_of(blocks, h0)
                        vector.wait_ge(s_mm, 2 * (c + 1))
                        vector.tensor_copy(
                            out=obs[p][0][bi][:, h0 - blo : h0 - blo + RCHUNK, :],
                            in_=banks[c % 4][0][:],
                        ).then_inc(s_cp, 1)
                        cp += 1
                for mc, bank in enumerate([minib[0], banks[1][0][:, 0:4, :]]):
                    bi = len(OBLOCKS1) + mc
                    vector.wait_ge(s_mm, 26 + 2 * (mc + 1))
                    vector.tensor_copy(out=obs[1][0][bi][:], in_=bank).then_inc(s_cp, 1)
                    cp += 1

    return nc


def _prep_inputs(x, K, mm_dt=MM_DT):
    np_dt = mybir.dt.np(mm_dt)
    x = np.ascontiguousarray(np.asarray(x, dtype=np.float32))
    K = np.ascontiguousarray(np.asarray(K, dtype=np.float32))
    xpad = np.pad(x, ((0, 0), (0, 0), (1, 1), (1, 1))).astype(np_dt)
    Wt = K.reshape(KS * KS * C_IN, C_OUT).reshape(C_IN, KS * KS, C_OUT)
    Wrep = np.ascontiguousarray(np.concatenate([Wt, Wt], axis=0)).astype(np_dt)
    shards = xpad.reshape(N_CORES, BPC, C_IN, HP, HP)
    return [{"x": np.ascontiguousarray(shards[i]), "w": Wrep} for i in range(N_CORES)]


def run(x, K, trace=False, mm_dt=MM_DT):
    nc = build_nc(mm_dt)
    in_maps = _prep_inputs(x, K, mm_dt)
    res = run_bass_kernel_spmd(nc, in_maps, list(range(N_CORES)), trace=trace)
    out = np.concatenate([res.results[i]["out"] for i in range(N_CORES)], axis=0)
    return out.astype(np.float32), res


def kernel(x, K):
    out, _ = run(x, K, trace=False)
    return out
